# revision 1
# baseline (speedup 1.0000x reference)
"""Trainium2 Bass kernel for nn_CCR_59193239273568 (3-pass spatial attention block).

Strategy (8 NeuronCores, SPMD):
  - Each core owns an 8-image-row band (512 px) of BOTH samples.
  - Phase A: per-band double-conv q/k/v projections (im2col 9-tap fp32r matmuls),
    band outputs + their TensorE-transposed form (ones column folded in for the
    softmax row-sum) are AllGathered per sample.
  - Phase B: per (sample, pass) unit the core computes its 512 query rows of
    softmax(scale * Q^T K) V^T in S^T layout: S^T chunks [128 keys, 512 queries]
    as K=32 fp32r matmuls, exp on ScalarE straight out of PSUM (scale folded
    into the activation), ctx^T + rowsum accumulated with a single M=33
    stationary operand (V^T | ones).  Normalization via DVE reciprocal (from
    SBUF) + GpSimd partition_broadcast.  Normalized ctx bands are AllGathered
    per sample.
  - Phase C: ctx relinearized to HBM, each core reads its band +3-row halo via a
    partition_id dynamic slice, runs the wr/wg/wb convs, averages (1/3 folded
    into w2's ctx input channels host-side), concat with x, w2, w3, writes its
    output band.  SAME-padding edge effects are fixed with per-core host "bias
    images" (-1e30 on out-of-image rows, so the conv relu zeroes them).
"""

import sys

import numpy as np

sys.path.insert(0, "/opt/trn_rl_repo")

import concourse.bacc as bacc
import concourse.bass as bass
import concourse.mybir as mybir
import concourse.tile as tile
from concourse.bass_utils import run_bass_kernel_spmd

F32 = mybir.dt.float32
F32R = mybir.dt.float32r
AF = mybir.ActivationFunctionType
ALU = mybir.AluOpType

B, CIN, C, H, W = 2, 64, 32, 64, 64
R = 8                 # cores
BR = H // R           # 8 band rows per core per sample
PX = BR * W           # 512 band pixels
N = H * W             # 4096
SCALE = float(C) ** -0.5
NCH = N // 128        # 32 key chunks per sample
GS = 3                # exp group size in chunks (PSUM: 2x3 banks + 1 ctx bank)

A_SZ = C * PX             # 16384  band in [C, PX] orientation
B_SZ = 128 * 132          # band^T chunks [128, 4, 33]: cols 0:32=V^T, col 32=ones
B_OFF = 3 * A_SZ
CONTRIB1 = 3 * A_SZ + 3 * B_SZ   # per-sample phase-A contribution (f32 elems)

# per-conv metadata: (weight pack name, cin, bias column)
CONVS = {
    "q1": ("wq1", 64, 0), "q2": ("wq2", 32, 1),
    "k1": ("wk1", 64, 2), "k2": ("wk2", 32, 3),
    "v1": ("wv1", 64, 4), "v2": ("wv2", 32, 5),
    "r": ("wr", 32, 6), "g": ("wg", 32, 7), "b": ("wb", 32, 8),
    "2": ("w2", 96, 9), "3": ("w3", 32, 10),
}


import os
_TIMING_ONLY = os.environ.get("TIMING_ONLY", "0") == "1"


def build_program():
    nc = bacc.Bacc("TRN2", target_bir_lowering=False, debug=False, num_devices=R)

    xband_d = nc.declare_dram_parameter("xband", [CIN, B, 12, 66], F32, isOutput=False)
    wd = {}
    for key, (wname, cin, _bi) in CONVS.items():
        wd[key] = nc.declare_dram_parameter("p_" + wname, [cin, 9, C], F32, isOutput=False)
    bias_d = nc.declare_dram_parameter("biases", [C, 11], F32, isOutput=False)
    biasA_d = nc.declare_dram_parameter("biasA", [C, 3, 10, W], F32, isOutput=False)
    biasC_d = nc.declare_dram_parameter("biasC", [C, 3, 12, W], F32, isOutput=False)
    biasD_d = nc.declare_dram_parameter("biasD", [C, 10, W], F32, isOutput=False)
    ident_d = nc.declare_dram_parameter("ident", [32, 32], F32, isOutput=False)
    out_d = nc.declare_dram_parameter("out", [B, C, BR, W], F32, isOutput=True)

    rg = [list(range(R))]

    with tile.TileContext(nc) as tc:
        with (
            tc.tile_pool(name="const", bufs=1) as constp,
            tc.tile_pool(name="persist", bufs=1) as persistp,
            tc.tile_pool(name="kv", bufs=2) as kvp,
            tc.tile_pool(name="exp", bufs=3) as ep,
            tc.tile_pool(name="small", bufs=2) as smallp,
            tc.tile_pool(name="phc", bufs=1) as phcp,
            tc.tile_pool(name="psum_s", bufs=2, space="PSUM") as psum_s,
            tc.tile_pool(name="psum_ctx", bufs=2, space="PSUM") as psum_ctx,
            tc.tile_pool(name="dram", bufs=1, space="DRAM") as dramp,
        ):
            pid = nc.sync.partition_id()

            _conv_ps_state = [0]

            def conv_psum(shape):
                _conv_ps_state[0] ^= 1
                if _conv_ps_state[0]:
                    return psum_s.tile(shape, F32, tag="s", name="cps")
                return psum_ctx.tile(shape, F32, tag="ctx", name="cps")

            # ---------------- constants into SBUF ----------------
            w_sb = {}
            for key, (wname, cin, _bi) in CONVS.items():
                t = constp.tile([cin, 9, C], F32R, tag="w" + key)
                nc.sync.dma_start(t[:], wd[key][:].bitcast(F32R))
                w_sb[key] = t
            bias_sb = constp.tile([C, 11], F32, tag="bias")
            nc.sync.dma_start(bias_sb[:], bias_d[:])
            biasA_sb = constp.tile([C, 3, 10, W], F32, tag="biasA")
            nc.sync.dma_start(biasA_sb[:], biasA_d[:])
            biasC_sb = constp.tile([C, 3, 12, W], F32, tag="biasC")
            nc.sync.dma_start(biasC_sb[:], biasC_d[:])
            biasD_sb = constp.tile([C, 10, W], F32, tag="biasD")
            nc.sync.dma_start(biasD_sb[:], biasD_d[:])
            ident_sb = constp.tile([32, 32], F32R, tag="ident")
            nc.sync.dma_start(ident_sb[:], ident_d[:].bitcast(F32R))
            zero_sb = constp.tile([C, 3 * W], F32, tag="zero")
            nc.vector.memset(zero_sb[:], 0.0)

            # warm the exp table early (overlaps with phase A)
            dummy = constp.tile([1, 16], F32, tag="dummy")
            nc.vector.memset(dummy[:], 0.0)
            nc.scalar.activation(dummy[:], dummy[:], AF.Exp)

            xband_sb = constp.tile([CIN, B, 12, 66], F32R, tag="xband")
            nc.sync.dma_start(xband_sb[:], xband_d[:].bitcast(F32R))

            def relu_bias(out_ap, psum_ap, bcol):
                # out = max(psum + bias[bcol], 0)
                nc.vector.tensor_scalar(
                    out_ap, psum_ap, bias_sb[:, bcol:bcol + 1], 0.0,
                    ALU.add, ALU.max,
                )

            def relu_img(out_ap, psum_ap, bimg_ap, tmp_tag):
                # out = max(psum + bias_image, 0) — bias image carries -1e30 on
                # out-of-image rows so the relu zeroes them.
                tmpb = smallp.tile(list(psum_ap.shape), F32, tag=tmp_tag, name="tmpb")
                nc.vector.tensor_add(tmpb[:], psum_ap, bimg_ap)
                nc.vector.tensor_scalar(out_ap, tmpb[:], 0.0, None, ALU.max)

            # ---------------- collective buffers ----------------
            contrib1 = [
                dramp.tile([CONTRIB1], F32, tag=f"c1_{s}", name=f"contrib1_{s}")
                for s in range(B)
            ]
            gath1 = [
                dramp.tile(
                    [R, CONTRIB1], F32, tag=f"g1_{s}", name=f"gath1_{s}",
                    addr_space="Local" if _TIMING_ONLY else "Shared",
                )
                for s in range(B)
            ]
            contrib2 = [
                dramp.tile([3 * A_SZ], F32, tag=f"c2_{s}", name=f"contrib2_{s}")
                for s in range(B)
            ]
            gath2 = [
                dramp.tile(
                    [R, 3 * A_SZ], F32, tag=f"g2_{s}", name=f"gath2_{s}",
                    addr_space="Local" if _TIMING_ONLY else "Shared",
                )
                for s in range(B)
            ]
            ctxlin = [
                [
                    dramp.tile([C, 70, W], F32, tag=f"cl_{s}_{p}", name=f"ctxlin_{s}_{p}")
                    for p in range(3)
                ]
                for s in range(B)
            ]
            # zero the 3-row guard bands of every ctxlin
            for s in range(B):
                for p in range(3):
                    nc.sync.dma_start(
                        ctxlin[s][p][:, 0:3, :].rearrange("c a w -> c (a w)"), zero_sb[:]
                    )
                    nc.sync.dma_start(
                        ctxlin[s][p][:, 67:70, :].rearrange("c a w -> c (a w)"), zero_sb[:]
                    )

            # ---------------- phase A: q/k/v bands ----------------
            qband = {}    # (s, t) -> [C, PX] SBUF
            for s in range(B):
                for t, tn in enumerate(("q", "k", "v")):
                    q1pad = persistp.tile([C, 10, 66], F32R, tag=f"q1pad_{s}_{t}")
                    zsrc = zero_sb[:, 0:10].rearrange("c (a b) -> c a b", b=1).bitcast(F32R)
                    nc.sync.dma_start(q1pad[:, :, 0:1], zsrc)
                    nc.sync.dma_start(q1pad[:, :, 65:66], zsrc)
                    for j0 in (0, 5):
                        ps = conv_psum([C, 5, W])
                        for tap in range(9):
                            dy, dx = divmod(tap, 3)
                            nc.tensor.matmul(
                                ps[:],
                                w_sb[tn + "1"][:, tap, :],
                                xband_sb[:, s, j0 + dy:j0 + dy + 5, dx:dx + W],
                                start=(tap == 0), stop=(tap == 8),
                            )
                        relu_img(
                            q1pad[:, j0:j0 + 5, 1:65], ps[:],
                            biasA_sb[:, t, j0:j0 + 5, :], "tmpA",
                        )

                    _, _, bi2 = CONVS[tn + "2"]
                    ps = conv_psum([C, BR, W])
                    for tap in range(9):
                        dy, dx = divmod(tap, 3)
                        nc.tensor.matmul(
                            ps[:],
                            w_sb[tn + "2"][:, tap, :],
                            q1pad[:, dy:dy + BR, dx:dx + W],
                            start=(tap == 0), stop=(tap == 8),
                        )
                    qb = persistp.tile([C, BR, W], F32R, tag=f"qband_{s}_{t}")
                    relu_bias(qb[:], ps[:], bi2)
                    qband[(s, t)] = qb

                    # transposed band chunks [128, 4, 64]: col 0 = ones,
                    # cols 32:64 = V^T (rowsum lands in psum row 0, ctx in 32:63)
                    vtb = persistp.tile([128, 4, 33], F32, tag=f"vtb_{s}_{t}")
                    nc.vector.memset(vtb[:, :, 32:33], 1.0)
                    qbf = qb[:].rearrange("c a w -> c (a w)")
                    for ii in range(4):
                        trp = psum_s.tile([128, 32], F32R, tag="s", name="trp")
                        nc.tensor.transpose(
                            trp[:], qbf[:, 128 * ii:128 * ii + 128], ident_sb[:]
                        )
                        nc.vector.tensor_copy(vtb[:, ii, 0:32], trp[:].bitcast(F32))

                    nc.sync.dma_start(
                        contrib1[s][t * A_SZ:(t + 1) * A_SZ]
                        .rearrange("(c a w) -> c a w", c=C, w=W).bitcast(F32R),
                        qb[:],
                    )
                    nc.sync.dma_start(
                        contrib1[s][B_OFF + t * B_SZ:B_OFF + (t + 1) * B_SZ]
                        .rearrange("(p a w) -> p a w", p=128, a=4, w=33),
                        vtb[:],
                    )

                if _TIMING_ONLY:
                    for _r in range(R):
                        nc.sync.dma_start(gath1[s][_r], contrib1[s][:])
                else:
                    nc.gpsimd.collective_compute(
                        "AllGather", ALU.bypass, replica_groups=rg,
                        ins=[contrib1[s].opt()], outs=[gath1[s].opt()],
                    )

            # ---------------- phase B: attention units ----------------
            for s in range(B):
                for p in range(3):
                    qt, kt, vt = p, (p + 1) % 3, (p + 2) % 3

                    ksb = kvp.tile([C, R, PX], F32R, tag="ksb")
                    nc.sync.dma_start(
                        ksb[:],
                        gath1[s][:, kt * A_SZ:(kt + 1) * A_SZ]
                        .rearrange("g (c px) -> c g px", c=C).bitcast(F32R),
                    )
                    vtsb = kvp.tile([128, R, 4, 33], F32R, tag="vtsb")
                    nc.sync.dma_start(
                        vtsb[:],
                        gath1[s][:, B_OFF + vt * B_SZ:B_OFF + (vt + 1) * B_SZ]
                        .rearrange("g (p a w) -> p g a w", p=128, a=4, w=33).bitcast(F32R),
                    )
                    qrhs = qband[(s, qt)][:].rearrange("c a w -> c (a w)")

                    ctxps = psum_ctx.tile([128, PX], F32, tag="ctx")
                    ngroups = (NCH + GS - 1) // GS

                    def emit_s_group(g):
                        csz = min(GS, NCH - g * GS)
                        sps = psum_s.tile([128, GS * PX], F32, tag="s", name="sps")
                        for ci in range(csz):
                            i = g * GS + ci
                            rr, ip = divmod(i, 4)
                            nc.tensor.matmul(
                                sps[:, ci * PX:(ci + 1) * PX],
                                ksb[:, rr, 128 * ip:128 * ip + 128],
                                qrhs,
                                start=True, stop=True,
                            )
                        return sps, csz

                    # software pipeline: emit S(g+1) before ctx(g) so the PE
                    # stream never blocks on exp(g) before starting S(g+1)
                    sps, csz = emit_s_group(0)
                    for g in range(ngroups):
                        es = ep.tile([128, GS * PX], F32R, tag="e")
                        nc.scalar.activation(
                            es[:, 0:csz * PX], sps[:, 0:csz * PX], AF.Exp, scale=SCALE
                        )
                        cur_csz = csz
                        if g + 1 < ngroups:
                            sps, csz = emit_s_group(g + 1)
                        for ci in range(cur_csz):
                            i = g * GS + ci
                            rr, ip = divmod(i, 4)
                            nc.tensor.matmul(
                                ctxps[0:33, :],
                                vtsb[:, rr, ip, :],
                                es[:, ci * PX:(ci + 1) * PX],
                                start=(i == 0), stop=(i == NCH - 1),
                            )

                    rs = smallp.tile([1, PX], F32, tag="rs")
                    nc.vector.tensor_copy(rs[:], ctxps[32:33, :])
                    recip = smallp.tile([1, PX], F32, tag="recip")
                    nc.vector.reciprocal(recip[:], rs[:])
                    bcast = smallp.tile([C, PX], F32, tag="bcast")
                    nc.gpsimd.partition_broadcast(bcast[:], recip[:])
                    ctxn = smallp.tile([C, PX], F32, tag="ctxn")
                    nc.vector.tensor_mul(ctxn[:], ctxps[0:32, :], bcast[:])
                    nc.sync.dma_start(
                        contrib2[s][p * A_SZ:(p + 1) * A_SZ].rearrange("(c f) -> c f", c=C),
                        ctxn[:],
                    )

                if _TIMING_ONLY:
                    for _r in range(R):
                        nc.sync.dma_start(gath2[s][_r], contrib2[s][:])
                else:
                    nc.gpsimd.collective_compute(
                        "AllGather", ALU.bypass, replica_groups=rg,
                        ins=[contrib2[s].opt()], outs=[gath2[s].opt()],
                    )

            # ---------------- phase C: output convs ----------------
            for s in range(B):
                # relinearize ctx rows (rank-major -> row-major with guards)
                for p in range(3):
                    nc.sync.dma_start(
                        ctxlin[s][p][:, 3:67, :].rearrange("c (g j) w -> c g (j w)", g=R),
                        gath2[s][:, p * A_SZ:(p + 1) * A_SZ].rearrange("g (c f) -> c g f", c=C),
                    )

                tmp = {}
                for p, pn in enumerate(("r", "g", "b")):
                    cpad = phcp.tile([C, 14, 66], F32R, tag="cpad")
                    zsrc14 = zero_sb[:, 0:14].rearrange("c (a b) -> c a b", b=1).bitcast(F32R)
                    nc.sync.dma_start(cpad[:, :, 0:1], zsrc14)
                    nc.sync.dma_start(cpad[:, :, 65:66], zsrc14)
                    nc.sync.dma_start(
                        cpad[:, :, 1:65],
                        ctxlin[s][p][:, bass.ds(pid * BR, 14), :].bitcast(F32R),
                    )
                    tp = phcp.tile([C, 12, W], F32, tag=f"tmp{p}")
                    for j0 in (0, 6):
                        ps = conv_psum([C, 6, W])
                        for tap in range(9):
                            dy, dx = divmod(tap, 3)
                            nc.tensor.matmul(
                                ps[:],
                                w_sb[pn][:, tap, :],
                                cpad[:, j0 + dy:j0 + dy + 6, dx:dx + W],
                                start=(tap == 0), stop=(tap == 8),
                            )
                        relu_img(
                            tp[:, j0:j0 + 6, :], ps[:],
                            biasC_sb[:, p, j0:j0 + 6, :], "tmpC",
                        )
                    tmp[p] = tp

                xctx = phcp.tile([96, 12, 66], F32R, tag="xctx")
                zsrc12 = zero_sb[:, 0:12].rearrange("c (a b) -> c a b", b=1).bitcast(F32R)
                nc.sync.dma_start(xctx[64:96, :, 0:1], zsrc12)
                nc.sync.dma_start(xctx[64:96, :, 65:66], zsrc12)
                nc.sync.dma_start(xctx[0:64, :, :], xband_sb[:, s, :, :])
                avg = phcp.tile([C, 12, W], F32, tag="avg")
                nc.vector.tensor_add(avg[:], tmp[0][:], tmp[1][:])
                avg2 = phcp.tile([C, 12, W], F32, tag="avg2")
                nc.vector.tensor_add(avg2[:], avg[:], tmp[2][:])
                nc.sync.dma_start(xctx[64:96, :, 1:65], avg2[:].bitcast(F32R))

                w2pad = phcp.tile([C, 10, 66], F32R, tag="w2pad")
                zsrc10 = zero_sb[:, 0:10].rearrange("c (a b) -> c a b", b=1).bitcast(F32R)
                nc.sync.dma_start(w2pad[:, :, 0:1], zsrc10)
                nc.sync.dma_start(w2pad[:, :, 65:66], zsrc10)
                for j0 in (0, 5):
                    ps = conv_psum([C, 5, W])
                    for tap in range(9):
                        dy, dx = divmod(tap, 3)
                        nc.tensor.matmul(
                            ps[:],
                            w_sb["2"][:, tap, :],
                            xctx[:, j0 + dy:j0 + dy + 5, dx:dx + W],
                            start=(tap == 0), stop=(tap == 8),
                        )
                    relu_img(
                        w2pad[:, j0:j0 + 5, 1:65], ps[:],
                        biasD_sb[:, j0:j0 + 5, :], "tmpD",
                    )

                ps = conv_psum([C, BR, W])
                _, _, bi3 = CONVS["3"]
                for tap in range(9):
                    dy, dx = divmod(tap, 3)
                    nc.tensor.matmul(
                        ps[:],
                        w_sb["3"][:, tap, :],
                        w2pad[:, dy:dy + BR, dx:dx + W],
                        start=(tap == 0), stop=(tap == 8),
                    )
                outsb = smallp.tile([C, BR, W], F32, tag="outsb")
                relu_bias(outsb[:], ps[:], bi3)
                nc.sync.dma_start(out_d[s], outsb[:])

    nc.compile()
    return nc


def _pack_w(w):
    # [Cout, Cin, 3, 3] -> lhsT pack [Cin, 9, Cout]
    w = np.asarray(w, np.float32)
    return np.ascontiguousarray(w.transpose(1, 2, 3, 0).reshape(w.shape[1], 9, w.shape[0]))


NEG = np.float32(-1e30)


def prep_in_maps(inputs):
    x = np.asarray(inputs["x"], np.float32)
    xp = np.zeros((B, CIN, H + 4, W + 2), np.float32)
    xp[:, :, 2:2 + H, 1:1 + W] = x

    shared = {}
    for key, (wname, cin, _bi) in CONVS.items():
        w = np.asarray(inputs[wname], np.float32)
        if key == "2":
            w = w.copy()
            w[:, CIN:, :, :] /= 3.0   # fold the ctx 3-way average into w2
        shared["p_" + wname] = _pack_w(w)
    bnames = ("bq1", "bq2", "bk1", "bk2", "bv1", "bv2", "br", "bg", "bb", "b2", "b3")
    bvals = {bn: np.asarray(inputs[bn], np.float32) for bn in bnames}
    shared["biases"] = np.ascontiguousarray(np.stack([bvals[bn] for bn in bnames], axis=1))
    shared["ident"] = np.eye(32, dtype=np.float32)

    in_maps = []
    for r in range(R):
        r0 = BR * r
        xband = np.ascontiguousarray(
            xp[:, :, r0:r0 + 12, :].transpose(1, 0, 2, 3)
        )  # [CIN, B, 12, 66]

        # bias images; -1e30 rows get relu'd to the zero SAME padding expects
        biasA = np.stack(
            [np.broadcast_to(bvals[bn][:, None, None], (C, 10, W)).copy()
             for bn in ("bq1", "bk1", "bv1")], axis=1,
        )  # [C, 3, 10, W] ; conv1 out rows r0-1 .. r0+8
        biasC = np.stack(
            [np.broadcast_to(bvals[bn][:, None, None], (C, 12, W)).copy()
             for bn in ("br", "bg", "bb")], axis=1,
        )  # [C, 3, 12, W] ; wr/g/b out rows r0-2 .. r0+9
        biasD = np.broadcast_to(bvals["b2"][:, None, None], (C, 10, W)).copy()
        if r == 0:
            biasA[:, :, 0, :] = NEG
            biasC[:, :, 0:2, :] = NEG
            biasD[:, 0, :] = NEG
        if r == R - 1:
            biasA[:, :, 9, :] = NEG
            biasC[:, :, 10:12, :] = NEG
            biasD[:, 9, :] = NEG

        in_maps.append(dict(
            shared, xband=xband,
            biasA=np.ascontiguousarray(biasA),
            biasC=np.ascontiguousarray(biasC),
            biasD=np.ascontiguousarray(biasD),
        ))
    return in_maps


_CACHE = {}


def get_program():
    if "nc" not in _CACHE:
        _CACHE["nc"] = build_program()
    return _CACHE["nc"]


def kernel(**inputs):
    nc = get_program()
    in_maps = prep_in_maps(inputs)
    res = run_bass_kernel_spmd(nc, in_maps, list(range(R)))
    out = np.zeros((B, C, H, W), np.float32)
    for r in range(R):
        out[:, :, BR * r:BR * (r + 1), :] = res.results[r]["out"]
    return out



# revision 16
# speedup vs baseline: 1.8947x; 1.8947x over previous
"""Trainium2 Bass kernel for nn_CCR_59193239273568 (3-pass spatial attention block).

Strategy (8 NeuronCores, SPMD), v2:
  - Each core owns an 8-image-row band (512 px) of BOTH samples.
  - Phase A: q/k/v double-conv projections computed STACKED: conv1 emits all
    three 32-ch maps in one 96-partition pass; conv2 uses a block-diagonal
    96x96 weight pack.  Band outputs (bf16) are AllGathered per sample
    (A-form only; the key-major transposed form is rebuilt locally after the
    gather with PE transposes, which is cheaper than shipping it).
  - Phase B: per (sample, pass) unit the core computes its 512 query rows:
    S^T chunks [128 keys, 512 q] as K=32 bf16 matmuls, exp on ScalarE with
    the softmax scale folded in (bf16 output), then the context matmul is
    FLIPPED: exp(S^T) 128x128 blocks are the stationary operand and
    V^T|ones [128, 33] the moving one, so the PE streams 33 rows instead of
    512 per chunk.  The rowsum lands per query partition, so normalization
    is a plain per-partition tensor_scalar (no partition broadcast).
    Normalized ctx is transposed back to channel-major and AllGathered.
  - Phase C: each core reads the gathered ctx (96-partition pass-stacked)
    via a partition_id dynamic row slice directly in the conv rhs (no DRAM
    relinearize), runs wr/wg/wb as one block-diagonal 96x96 conv, averages
    (1/3 folded into w2), concats with x, w2, w3, writes its output band.
    SAME-padding column edges are handled with shifted-column psum
    accumulation (no padded copies); row edges via per-core host bias
    images (-1e30 rows that the conv relu zeroes).
"""

import sys

import numpy as np

sys.path.insert(0, "/opt/trn_rl_repo")

import concourse.bacc as bacc
import concourse.bass as bass
import concourse.mybir as mybir
import concourse.tile as tile
from concourse.bass_utils import run_bass_kernel_spmd

F32 = mybir.dt.float32
F32R = mybir.dt.float32r
BF16 = mybir.dt.bfloat16
AF = mybir.ActivationFunctionType
ALU = mybir.AluOpType

B, CIN, C, H, W = 2, 64, 32, 64, 64
R = 8                 # cores
BR = H // R           # 8 band rows per core per sample
PX = BR * W           # 512 band pixels
N = H * W             # 4096
SCALE = float(C) ** -0.5
NCH = N // 128        # 32 key chunks per sample
GS = 3                # exp group size in chunks (PSUM: 2x3 banks + 2 misc)
A_SZ = C * PX         # 16384 band elems in [C, PX] orientation


TAPS = [(dy, dx) for dy in range(3) for dx in range(3)]
A2_SZ = C * BR * 66       # 16896: ctx band with 66-wide padded rows


def build_program():
    nc = bacc.Bacc("TRN2", target_bir_lowering=False, debug=False, num_devices=R)

    xband_d = nc.declare_dram_parameter("xband", [CIN, B, 12, 66], BF16, isOutput=False)
    wA_d = nc.declare_dram_parameter("wA", [CIN, 9, 96], BF16, isOutput=False)
    wB_d = nc.declare_dram_parameter("wB", [96, 9, 96], BF16, isOutput=False)
    wC_d = nc.declare_dram_parameter("wC", [96, 9, 96], BF16, isOutput=False)
    w2_d = nc.declare_dram_parameter("w2", [96, 9, C], BF16, isOutput=False)
    w3_d = nc.declare_dram_parameter("w3", [C, 9, C], BF16, isOutput=False)
    ident_d = nc.declare_dram_parameter("ident", [128, 128], BF16, isOutput=False)
    sel3_d = nc.declare_dram_parameter("sel3", [96, C], BF16, isOutput=False)
    biasQ_d = nc.declare_dram_parameter("biasQ", [96, 1], F32, isOutput=False)
    bias3_d = nc.declare_dram_parameter("bias3", [C, 1], F32, isOutput=False)
    biasA_d = nc.declare_dram_parameter("biasA", [96, 10, W], F32, isOutput=False)
    biasC_d = nc.declare_dram_parameter("biasC", [96, 12, W], F32, isOutput=False)
    biasD_d = nc.declare_dram_parameter("biasD", [C, 10, W], F32, isOutput=False)
    out_d = nc.declare_dram_parameter("out", [B, C, BR, W], F32, isOutput=True)

    rg = [list(range(R))]

    with tile.TileContext(nc) as tc:
        with (
            tc.tile_pool(name="const", bufs=1) as constp,
            tc.tile_pool(name="persist", bufs=1) as persistp,
            tc.tile_pool(name="kv", bufs=2) as kvp,
            tc.tile_pool(name="exp", bufs=3) as ep,
            tc.tile_pool(name="small", bufs=2) as smallp,
            tc.tile_pool(name="phc", bufs=1) as phcp,
            tc.tile_pool(name="psum_s", bufs=2, space="PSUM") as psum_s,
            tc.tile_pool(name="psum_m", bufs=2, space="PSUM") as psum_m,
            tc.tile_pool(name="dram", bufs=1, space="DRAM") as dramp,
        ):
            pid = nc.partition_id()   # register on every engine (PE reads it)

            # ---------------- constants into SBUF ----------------
            wA = constp.tile([CIN, 9, 96], BF16, tag="wA")
            nc.sync.dma_start(wA[:], wA_d[:])
            wB = constp.tile([96, 9, 96], BF16, tag="wB")
            nc.sync.dma_start(wB[:], wB_d[:])
            wC = constp.tile([96, 9, 96], BF16, tag="wC")
            nc.sync.dma_start(wC[:], wC_d[:])
            w2 = constp.tile([96, 9, C], BF16, tag="w2")
            nc.sync.dma_start(w2[:], w2_d[:])
            w3 = constp.tile([C, 9, C], BF16, tag="w3")
            nc.sync.dma_start(w3[:], w3_d[:])
            ident = constp.tile([128, 128], BF16, tag="ident")
            nc.sync.dma_start(ident[:], ident_d[:])
            sel3 = constp.tile([96, C], BF16, tag="sel3")
            nc.sync.dma_start(sel3[:], sel3_d[:])
            biasQ = constp.tile([96, 1], F32, tag="biasQ")
            nc.sync.dma_start(biasQ[:], biasQ_d[:])
            bias3 = constp.tile([C, 1], F32, tag="bias3")
            nc.sync.dma_start(bias3[:], bias3_d[:])
            biasA = constp.tile([96, 10, W], F32, tag="biasA")
            nc.sync.dma_start(biasA[:], biasA_d[:])
            biasC = constp.tile([96, 12, W], F32, tag="biasC")
            nc.sync.dma_start(biasC[:], biasC_d[:])
            biasD = constp.tile([C, 10, W], F32, tag="biasD")
            nc.sync.dma_start(biasD[:], biasD_d[:])
            xband = constp.tile([CIN, B, 12, 66], BF16, tag="xband")
            nc.sync.dma_start(xband[:], xband_d[:])

            # warm the exp table early (overlaps with phase A)
            dummy = constp.tile([1, 16], F32, tag="dummy")
            nc.vector.memset(dummy[:], 0.0)
            nc.scalar.activation(dummy[:], dummy[:], AF.Exp)

            # ---------------- collective buffers ----------------
            contrib1 = [
                dramp.tile([3, A_SZ], BF16, tag=f"c1_{s}", name=f"contrib1_{s}")
                for s in range(B)
            ]
            gath1 = [
                dramp.tile([R, 3, A_SZ], BF16, tag=f"g1_{s}", name=f"gath1_{s}",
                           addr_space="Shared")
                for s in range(B)
            ]
            contrib2 = [
                dramp.tile([3, A2_SZ], BF16, tag=f"c2_{s}", name=f"contrib2_{s}")
                for s in range(B)
            ]
            gath2 = [
                dramp.tile([R, 3, A2_SZ], BF16, tag=f"g2_{s}", name=f"gath2_{s}",
                           addr_space="Shared")
                for s in range(B)
            ]

            _ps_state = [0]

            def conv_psum(shape, dtype=F32):
                _ps_state[0] ^= 1
                pool = psum_m if _ps_state[0] else psum_s
                return pool.tile(shape, dtype, tag="m" if _ps_state[0] else "s",
                                 name="cps")

            def relu_img(out_ap, psum_ap, bimg_ap, shape, tmp_tag):
                # out = max(psum + bias_image, 0); -1e30 rows relu to 0.
                tmpb = smallp.tile(shape, F32, tag=tmp_tag, name="tmpb")
                nc.vector.tensor_add(tmpb[:], psum_ap, bimg_ap)
                nc.vector.tensor_scalar(out_ap, tmpb[:], 0.0, None, ALU.max)

            # ---------------- phase A ----------------
            qball = {}
            for s in range(B):
                # conv1: all of q1|k1|v1 in one 96-wide pass, two row halves
                q1pad = persistp.tile([96, 10, 66], BF16, tag=f"q1pad_{s}")
                nc.vector.memset(q1pad[:, :, 0:1], 0.0)
                nc.vector.memset(q1pad[:, :, 65:66], 0.0)
                for j0 in (0, 5):
                    ps = conv_psum([96, 5, W])
                    for ti, (dy, dx) in enumerate(TAPS):
                        nc.tensor.matmul(
                            ps[:],
                            wA[:, 3 * dy + dx, :],
                            xband[:, s, j0 + dy:j0 + dy + 5, dx:dx + W],
                            start=(ti == 0), stop=(ti == 8),
                        )
                    relu_img(q1pad[:, j0:j0 + 5, 1:65], ps[:],
                             biasA[:, j0:j0 + 5, :], [96, 5, W], "tA")

                # conv2: block-diagonal 96x96, one 512-row pass
                ps = conv_psum([96, BR, W])
                for ti, (dy, dx) in enumerate(TAPS):
                    nc.tensor.matmul(
                        ps[:],
                        wB[:, 3 * dy + dx, :],
                        q1pad[:, dy:dy + BR, dx:dx + W],
                        start=(ti == 0), stop=(ti == 8),
                    )
                qb = persistp.tile([96, BR, W], BF16, tag=f"qball_{s}")
                nc.vector.tensor_scalar(qb[:], ps[:], biasQ[:, 0:1], 0.0,
                                        ALU.add, ALU.max)
                qball[s] = qb

                nc.sync.dma_start(
                    contrib1[s][:].rearrange("t (c a w) -> (t c) a w", c=C, a=BR),
                    qb[:],
                )
                nc.gpsimd.collective_compute(
                    "AllGather", ALU.bypass, replica_groups=rg,
                    ins=[contrib1[s].opt()], outs=[gath1[s].opt()],
                )

            # ---------------- phase B ----------------
            ctxt = {}

            def phase_b_prep(s):
                ksb = kvp.tile([C, 3, R, PX], BF16, tag=f"ksb_{s}")
                for t in range(3):
                    nc.sync.dma_start(
                        ksb[:, t, :, :],
                        gath1[s][:, t, :].rearrange("g (c px) -> c g px", c=C),
                    )
                vtsb = kvp.tile([128, 3, R, 4, 33], BF16, tag=f"vtsb_{s}")
                nc.vector.memset(vtsb[:, :, :, :, 32:33], 1.0)
                for t in (2, 0, 1):          # pass p consumes vt=(p+2)%3
                    for g in range(R):
                        trp = psum_m.tile([128, 4, C], BF16, tag="m", name="trp")
                        for ip in range(4):
                            nc.tensor.transpose(
                                trp[:, ip, :],
                                ksb[:, t, g, 128 * ip:128 * ip + 128],
                                ident[0:C, 0:C],
                            )
                        nc.vector.tensor_copy(vtsb[:, t, g, :, 0:32], trp[:])
                return ksb, vtsb

            def phase_b_unit(s, p, ksb, vtsb):
                qt, kt, vt = p, (p + 1) % 3, (p + 2) % 3
                qrhs = smallp.tile([C, PX], BF16, tag="qrhs")
                nc.vector.tensor_copy(
                    qrhs[:], qball[s][32 * qt:32 * qt + 32, :].rearrange(
                        "c a w -> c (a w)"))

                ctxps = psum_m.tile([128, 4, 33], F32, tag="m", name="ctxps")
                # 4 interleaved accumulation streams share this bank: hardware
                # accumulates per-cell, but start=True would zero the whole
                # 2KB region — so pre-zero once and accumulate with start=False.
                nc.vector.memset(ctxps[:], 0.0)
                ngroups = (NCH + GS - 1) // GS

                def emit_s_group(g):
                    csz = min(GS, NCH - g * GS)
                    sps = psum_s.tile([128, GS, PX], F32, tag="s", name="sps")
                    for ci in range(csz):
                        i = g * GS + ci
                        rr, ip = divmod(i, 4)
                        nc.tensor.matmul(
                            sps[:, ci, :],
                            ksb[:, kt, rr, 128 * ip:128 * ip + 128],
                            qrhs[:],
                            start=True, stop=True,
                        )
                    return sps, csz

                sps, csz = emit_s_group(0)
                for g in range(ngroups):
                    es = ep.tile([128, GS, PX], BF16, tag="e")
                    nc.scalar.activation(
                        es[:].rearrange("p a w -> p (a w)")[:, 0:csz * PX],
                        sps[:].rearrange("p a w -> p (a w)")[:, 0:csz * PX],
                        AF.Exp, scale=SCALE,
                    )
                    cur_csz = csz
                    if g + 1 < ngroups:
                        sps, csz = emit_s_group(g + 1)
                    for ci in range(cur_csz):
                        i = g * GS + ci
                        rr, ip = divmod(i, 4)
                        for qb_ in range(4):
                            nc.tensor.matmul(
                                ctxps[:, qb_, :],
                                es[:, ci, 128 * qb_:128 * qb_ + 128],
                                vtsb[:, vt, rr, ip, :],
                                start=False, stop=(i == NCH - 1),
                                skip_group_check=True,
                            )

                # normalize per query partition, transpose back to ch-major
                rs = smallp.tile([128, 4], F32, tag="rs")
                nc.vector.tensor_copy(
                    rs[:], ctxps[:, :, 32:33].rearrange("p a o -> p (a o)"))
                recip = smallp.tile([128, 4], F32, tag="recip")
                nc.vector.reciprocal(recip[:], rs[:])
                ctxn = smallp.tile([128, 4, C], BF16, tag="ctxn")
                for qb_ in range(4):
                    nc.vector.tensor_scalar(
                        ctxn[:, qb_, :], ctxps[:, qb_, 0:32],
                        recip[:, qb_:qb_ + 1], None, ALU.mult,
                    )
                trc = psum_s.tile([C, 4, 128], BF16, tag="s", name="trc")
                for qb_ in range(4):
                    nc.tensor.transpose(
                        trc[:, qb_, :], ctxn[:, qb_, :], ident[:],
                    )
                nc.vector.tensor_copy(
                    ctxt[s][:, p, :, 1:65],
                    trc[:].rearrange("c a q -> c (a q)").rearrange(
                        "c (j w) -> c j w", w=W))

            def phase_b_sample(s, passes):
                for p in passes:
                    phase_b_unit(s, p, *_prep[s])

            _prep = {}
            for s in range(B):
                ctxt[s] = persistp.tile([C, 3, BR, 66], BF16, tag=f"ctxt_{s}",
                                        name=f"ctxt_{s}")
                nc.vector.memset(ctxt[s][:, :, :, 0:1], 0.0)
                nc.vector.memset(ctxt[s][:, :, :, 65:66], 0.0)

            _prep[0] = phase_b_prep(0)
            phase_b_sample(0, (0, 1, 2))
            nc.sync.dma_start(
                contrib2[0][:].rearrange("t (c jw) -> c t jw", c=C),
                ctxt[0][:].rearrange("c t j w -> c t (j w)"))
            nc.gpsimd.collective_compute(
                "AllGather", ALU.bypass, replica_groups=rg,
                ins=[contrib2[0].opt()], outs=[gath2[0].opt()],
            )
            _prep[1] = phase_b_prep(1)
            phase_b_sample(1, (0, 1, 2))
            nc.sync.dma_start(
                contrib2[1][:].rearrange("t (c jw) -> c t jw", c=C),
                ctxt[1][:].rearrange("c t j w -> c t (j w)"))
            nc.gpsimd.collective_compute(
                "AllGather", ALU.bypass, replica_groups=rg,
                ins=[contrib2[1].opt()], outs=[gath2[1].opt()],
            )

            # ---------------- phase C ----------------
            for s in range(B):
                # gathered ctx, pass-stacked on partitions, 3 guard rows each side
                cf = phcp.tile([96, 70, 66], BF16, tag=f"cf_{s}")
                nc.vector.memset(cf[:, 0:3, :], 0.0)
                nc.vector.memset(cf[:, 67:70, :], 0.0)
                nc.sync.dma_start(
                    cf[:, 3:67, :].rearrange("pc (g j) w -> pc g (j w)", g=R),
                    gath2[s][:].rearrange("g t (c jw) -> (t c) g jw", c=C),
                )

                xctx = phcp.tile([96, 12, 66], BF16, tag="xctx")
                nc.vector.memset(xctx[:, :, 0:1], 0.0)
                nc.vector.memset(xctx[:, :, 65:66], 0.0)
                nc.vector.tensor_copy(xctx[0:CIN, :, :], xband[:, s, :, :])  # 66-wide incl pads

                # wr|wg|wb as one block-diagonal 96-wide conv, two row halves
                tmp = phcp.tile([96, 12, W], BF16, tag="tmpC")
                for j0 in (0, 6):
                    ps = conv_psum([96, 6, W])
                    for ti, (dy, dx) in enumerate(TAPS):
                        nc.tensor.matmul(
                            ps[:],
                            wC[:, 3 * dy + dx, :],
                            cf[:, bass.ds(pid * BR + j0 + dy, 6), dx:dx + W],
                            start=(ti == 0), stop=(ti == 8),
                        )
                    relu_img(tmp[:, j0:j0 + 6, :], ps[:],
                             biasC[:, j0:j0 + 6, :], [96, 6, W], "tC")

                # sum the three 32-partition groups of tmp via a selection matmul
                for j0 in (0, 6):
                    aps = conv_psum([C, 6, W])
                    nc.tensor.matmul(aps[:], sel3[:], tmp[:, j0:j0 + 6, :],
                                     start=True, stop=True)
                    nc.vector.tensor_copy(xctx[64:96, j0:j0 + 6, 1:65], aps[:])

                w2buf = phcp.tile([C, 10, 66], BF16, tag="w2buf")
                nc.vector.memset(w2buf[:, :, 0:1], 0.0)
                nc.vector.memset(w2buf[:, :, 65:66], 0.0)
                for j0 in (0, 5):
                    ps = conv_psum([C, 5, W])
                    for ti, (dy, dx) in enumerate(TAPS):
                        nc.tensor.matmul(
                            ps[:],
                            w2[:, 3 * dy + dx, :],
                            xctx[:, j0 + dy:j0 + dy + 5, dx:dx + W],
                            start=(ti == 0), stop=(ti == 8),
                        )
                    relu_img(w2buf[:, j0:j0 + 5, 1:65], ps[:],
                             biasD[:, j0:j0 + 5, :], [C, 5, W], "tD")

                ps = conv_psum([C, BR, W])
                for ti, (dy, dx) in enumerate(TAPS):
                    nc.tensor.matmul(
                        ps[:],
                        w3[:, 3 * dy + dx, :],
                        w2buf[:, dy:dy + BR, dx:dx + W],
                        start=(ti == 0), stop=(ti == 8),
                    )
                outsb = smallp.tile([C, BR, W], F32, tag="outsb")
                nc.vector.tensor_scalar(outsb[:], ps[:], bias3[:, 0:1], 0.0,
                                        ALU.add, ALU.max)
                nc.sync.dma_start(out_d[s], outsb[:])

    nc.compile()
    return nc


def _pack_w(w):
    # [Cout, Cin, 3, 3] -> lhsT pack [Cin, 9, Cout]
    w = np.asarray(w, np.float32)
    return np.ascontiguousarray(w.transpose(1, 2, 3, 0).reshape(w.shape[1], 9, w.shape[0]))


NEG = np.float32(-1e30)


def _bf16(a):
    import ml_dtypes
    return np.asarray(a, np.float32).astype(ml_dtypes.bfloat16)


def prep_in_maps(inputs):
    x = np.asarray(inputs["x"], np.float32)
    xp = np.zeros((B, CIN, H + 4, W + 2), np.float32)
    xp[:, :, 2:2 + H, 1:1 + W] = x

    shared = {}
    # wA: conv1 stacked [64, 9, 96] (q|k|v)
    shared["wA"] = _bf16(np.concatenate(
        [_pack_w(inputs[n]) for n in ("wq1", "wk1", "wv1")], axis=2))
    # wB: conv2 block-diagonal [96, 9, 96]
    wBb = np.zeros((96, 9, 96), np.float32)
    for t, n in enumerate(("wq2", "wk2", "wv2")):
        wBb[32 * t:32 * t + 32, :, 32 * t:32 * t + 32] = _pack_w(inputs[n])
    shared["wB"] = _bf16(wBb)
    # wC: wr|wg|wb block-diagonal [96, 9, 96] bf16
    wCb = np.zeros((96, 9, 96), np.float32)
    for t, n in enumerate(("wr", "wg", "wb")):
        wCb[32 * t:32 * t + 32, :, 32 * t:32 * t + 32] = _pack_w(inputs[n])
    shared["wC"] = _bf16(wCb)
    w2v = np.asarray(inputs["w2"], np.float32).copy()
    w2v[:, CIN:, :, :] /= 3.0   # fold the ctx 3-way average into w2
    shared["w2"] = _bf16(_pack_w(w2v))
    shared["w3"] = _bf16(_pack_w(inputs["w3"]))
    shared["ident"] = _bf16(np.eye(128, dtype=np.float32))
    shared["sel3"] = _bf16(np.tile(np.eye(C, dtype=np.float32), (3, 1)))
    shared["biasQ"] = np.concatenate(
        [np.asarray(inputs[n], np.float32) for n in ("bq2", "bk2", "bv2")]
    ).reshape(96, 1)
    shared["bias3"] = np.asarray(inputs["b3"], np.float32).reshape(C, 1)

    bvals = {n: np.asarray(inputs[n], np.float32)
             for n in ("bq1", "bk1", "bv1", "br", "bg", "bb", "b2")}

    in_maps = []
    for r in range(R):
        r0 = BR * r
        xbandv = np.ascontiguousarray(
            xp[:, :, r0:r0 + 12, :].transpose(1, 0, 2, 3)
        )  # [CIN, B, 12, 66]

        # bias images; -1e30 rows get relu'd to the zero SAME padding expects
        biasA = np.concatenate(
            [np.broadcast_to(bvals[n][:, None, None], (C, 10, W)).copy()
             for n in ("bq1", "bk1", "bv1")], axis=0)   # [96, 10, W]
        biasCv = np.concatenate(
            [np.broadcast_to(bvals[n][:, None, None], (C, 12, W)).copy()
             for n in ("br", "bg", "bb")], axis=0)      # [96, 12, W]
        biasD = np.broadcast_to(bvals["b2"][:, None, None], (C, 10, W)).copy()
        if r == 0:
            biasA[:, 0, :] = NEG
            biasCv[:, 0:2, :] = NEG
            biasD[:, 0, :] = NEG
        if r == R - 1:
            biasA[:, 9, :] = NEG
            biasCv[:, 10:12, :] = NEG
            biasD[:, 9, :] = NEG

        in_maps.append(dict(
            shared, xband=_bf16(xbandv),
            biasA=np.ascontiguousarray(biasA),
            biasC=np.ascontiguousarray(biasCv),
            biasD=np.ascontiguousarray(biasD),
        ))
    return in_maps


_CACHE = {}


def get_program():
    if "nc" not in _CACHE:
        _CACHE["nc"] = build_program()
    return _CACHE["nc"]


def kernel(**inputs):
    nc = get_program()
    in_maps = prep_in_maps(inputs)
    res = run_bass_kernel_spmd(nc, in_maps, list(range(R)))
    out = np.zeros((B, C, H, W), np.float32)
    for r in range(R):
        out[:, :, BR * r:BR * (r + 1), :] = res.results[r]["out"]
    return out


# revision 18
# speedup vs baseline: 1.9650x; 1.0371x over previous
"""Trainium2 Bass kernel for nn_CCR_59193239273568 (3-pass spatial attention block).

Strategy (8 NeuronCores, SPMD), v2:
  - Each core owns an 8-image-row band (512 px) of BOTH samples.
  - Phase A: q/k/v double-conv projections computed STACKED: conv1 emits all
    three 32-ch maps in one 96-partition pass; conv2 uses a block-diagonal
    96x96 weight pack.  Band outputs (bf16) are AllGathered per sample
    (A-form only; the key-major transposed form is rebuilt locally after the
    gather with PE transposes, which is cheaper than shipping it).
  - Phase B: per (sample, pass) unit the core computes its 512 query rows:
    S^T chunks [128 keys, 512 q] as K=32 bf16 matmuls, exp on ScalarE with
    the softmax scale folded in (bf16 output), then the context matmul is
    FLIPPED: exp(S^T) 128x128 blocks are the stationary operand and
    V^T|ones [128, 33] the moving one, so the PE streams 33 rows instead of
    512 per chunk.  The rowsum lands per query partition, so normalization
    is a plain per-partition tensor_scalar (no partition broadcast).
    Normalized ctx is transposed back to channel-major and AllGathered.
  - Phase C: each core reads the gathered ctx (96-partition pass-stacked)
    via a partition_id dynamic row slice directly in the conv rhs (no DRAM
    relinearize), runs wr/wg/wb as one block-diagonal 96x96 conv, averages
    (1/3 folded into w2), concats with x, w2, w3, writes its output band.
    SAME-padding column edges are handled with shifted-column psum
    accumulation (no padded copies); row edges via per-core host bias
    images (-1e30 rows that the conv relu zeroes).
"""

import sys

import numpy as np

sys.path.insert(0, "/opt/trn_rl_repo")

import concourse.bacc as bacc
import concourse.bass as bass
import concourse.mybir as mybir
import concourse.tile as tile
from concourse.bass_utils import run_bass_kernel_spmd

F32 = mybir.dt.float32
F32R = mybir.dt.float32r
BF16 = mybir.dt.bfloat16
AF = mybir.ActivationFunctionType
ALU = mybir.AluOpType

B, CIN, C, H, W = 2, 64, 32, 64, 64
R = 8                 # cores
BR = H // R           # 8 band rows per core per sample
PX = BR * W           # 512 band pixels
N = H * W             # 4096
SCALE = float(C) ** -0.5
NCH = N // 128        # 32 key chunks per sample
GS = 3                # exp group size in chunks (PSUM: 2x3 banks + 2 misc)
A_SZ = C * PX         # 16384 band elems in [C, PX] orientation


TAPS = [(dy, dx) for dy in range(3) for dx in range(3)]
A2_SZ = C * BR * 66       # 16896: ctx band with 66-wide padded rows


def build_program():
    nc = bacc.Bacc("TRN2", target_bir_lowering=False, debug=False, num_devices=R)

    xband_d = nc.declare_dram_parameter("xband", [CIN, B, 12, 66], BF16, isOutput=False)
    wA_d = nc.declare_dram_parameter("wA", [CIN, 9, 96], BF16, isOutput=False)
    wB_d = nc.declare_dram_parameter("wB", [96, 9, 96], BF16, isOutput=False)
    wC_d = nc.declare_dram_parameter("wC", [96, 9, 96], BF16, isOutput=False)
    w2_d = nc.declare_dram_parameter("w2", [96, 9, C], BF16, isOutput=False)
    w3_d = nc.declare_dram_parameter("w3", [C, 9, C], BF16, isOutput=False)
    ident_d = nc.declare_dram_parameter("ident", [128, 128], BF16, isOutput=False)
    sel3_d = nc.declare_dram_parameter("sel3", [96, C], BF16, isOutput=False)
    biasQ_d = nc.declare_dram_parameter("biasQ", [96, 1], F32, isOutput=False)
    bias3_d = nc.declare_dram_parameter("bias3", [C, 1], F32, isOutput=False)
    biasA_d = nc.declare_dram_parameter("biasA", [96, 10, W], F32, isOutput=False)
    biasC_d = nc.declare_dram_parameter("biasC", [96, 12, W], F32, isOutput=False)
    biasD_d = nc.declare_dram_parameter("biasD", [C, 10, W], F32, isOutput=False)
    out_d = nc.declare_dram_parameter("out", [B, C, BR, W], F32, isOutput=True)

    rg = [list(range(R))]

    with tile.TileContext(nc) as tc:
        with (
            tc.tile_pool(name="const", bufs=1) as constp,
            tc.tile_pool(name="persist", bufs=1) as persistp,
            tc.tile_pool(name="kv", bufs=2) as kvp,
            tc.tile_pool(name="exp", bufs=3) as ep,
            tc.tile_pool(name="small", bufs=2) as smallp,
            tc.tile_pool(name="phc", bufs=1) as phcp,
            tc.tile_pool(name="psum_s", bufs=2, space="PSUM") as psum_s,
            tc.tile_pool(name="psum_m", bufs=2, space="PSUM") as psum_m,
            tc.tile_pool(name="dram", bufs=1, space="DRAM") as dramp,
        ):
            pid = nc.partition_id()   # register on every engine (PE reads it)

            # ---------------- constants into SBUF ----------------
            # phase-A-critical consts on the SP queue, the rest on the Act
            # queue so both DMA queues fill in parallel
            wA = constp.tile([CIN, 9, 96], BF16, tag="wA")
            nc.sync.dma_start(wA[:], wA_d[:])
            xband = constp.tile([CIN, B, 12, 66], BF16, tag="xband")
            nc.sync.dma_start(xband[:], xband_d[:])
            biasA = constp.tile([96, 10, W], F32, tag="biasA")
            nc.sync.dma_start(biasA[:], biasA_d[:])
            wB = constp.tile([96, 9, 96], BF16, tag="wB")
            nc.sync.dma_start(wB[:], wB_d[:])
            biasQ = constp.tile([96, 1], F32, tag="biasQ")
            nc.sync.dma_start(biasQ[:], biasQ_d[:])
            ident = constp.tile([128, 128], BF16, tag="ident")
            nc.scalar.dma_start(ident[:], ident_d[:])
            wC = constp.tile([96, 9, 96], BF16, tag="wC")
            nc.scalar.dma_start(wC[:], wC_d[:])
            w2 = constp.tile([96, 9, C], BF16, tag="w2")
            nc.scalar.dma_start(w2[:], w2_d[:])
            w3 = constp.tile([C, 9, C], BF16, tag="w3")
            nc.scalar.dma_start(w3[:], w3_d[:])
            sel3 = constp.tile([96, C], BF16, tag="sel3")
            nc.scalar.dma_start(sel3[:], sel3_d[:])
            bias3 = constp.tile([C, 1], F32, tag="bias3")
            nc.scalar.dma_start(bias3[:], bias3_d[:])
            biasC = constp.tile([96, 12, W], F32, tag="biasC")
            nc.scalar.dma_start(biasC[:], biasC_d[:])
            biasD = constp.tile([C, 10, W], F32, tag="biasD")
            nc.scalar.dma_start(biasD[:], biasD_d[:])

            # warm the exp table early (overlaps with phase A)
            dummy = constp.tile([1, 16], F32, tag="dummy")
            nc.vector.memset(dummy[:], 0.0)
            nc.scalar.activation(dummy[:], dummy[:], AF.Exp)

            # ---------------- collective buffers ----------------
            contrib1 = [
                dramp.tile([3, A_SZ], BF16, tag=f"c1_{s}", name=f"contrib1_{s}")
                for s in range(B)
            ]
            gath1 = [
                dramp.tile([R, 3, A_SZ], BF16, tag=f"g1_{s}", name=f"gath1_{s}",
                           addr_space="Shared")
                for s in range(B)
            ]
            contrib2 = [
                dramp.tile([3, A2_SZ], BF16, tag=f"c2_{s}", name=f"contrib2_{s}")
                for s in range(B)
            ]
            gath2 = [
                dramp.tile([R, 3, A2_SZ], BF16, tag=f"g2_{s}", name=f"gath2_{s}",
                           addr_space="Shared")
                for s in range(B)
            ]

            _ps_state = [0]

            def conv_psum(shape, dtype=F32):
                _ps_state[0] ^= 1
                pool = psum_m if _ps_state[0] else psum_s
                return pool.tile(shape, dtype, tag="m" if _ps_state[0] else "s",
                                 name="cps")

            def relu_img(out_ap, psum_ap, bimg_ap, shape, tmp_tag):
                # out = max(psum + bias_image, 0); -1e30 rows relu to 0.
                tmpb = smallp.tile(shape, F32, tag=tmp_tag, name="tmpb")
                nc.vector.tensor_add(tmpb[:], psum_ap, bimg_ap)
                nc.vector.tensor_scalar(out_ap, tmpb[:], 0.0, None, ALU.max)

            # ---------------- phase A ----------------
            qball = {}
            for s in range(B):
                # conv1: all of q1|k1|v1 in one 96-wide pass, two row halves
                q1pad = persistp.tile([96, 10, 66], BF16, tag=f"q1pad_{s}")
                nc.vector.memset(q1pad[:, :, 0:1], 0.0)
                nc.vector.memset(q1pad[:, :, 65:66], 0.0)
                for j0 in (0, 5):
                    ps = conv_psum([96, 5, W])
                    for ti, (dy, dx) in enumerate(TAPS):
                        nc.tensor.matmul(
                            ps[:],
                            wA[:, 3 * dy + dx, :],
                            xband[:, s, j0 + dy:j0 + dy + 5, dx:dx + W],
                            start=(ti == 0), stop=(ti == 8),
                        )
                    relu_img(q1pad[:, j0:j0 + 5, 1:65], ps[:],
                             biasA[:, j0:j0 + 5, :], [96, 5, W], "tA")

                # conv2: block-diagonal 96x96, one 512-row pass
                ps = conv_psum([96, BR, W])
                for ti, (dy, dx) in enumerate(TAPS):
                    nc.tensor.matmul(
                        ps[:],
                        wB[:, 3 * dy + dx, :],
                        q1pad[:, dy:dy + BR, dx:dx + W],
                        start=(ti == 0), stop=(ti == 8),
                    )
                qb = persistp.tile([96, BR, W], BF16, tag=f"qball_{s}")
                nc.vector.tensor_scalar(qb[:], ps[:], biasQ[:, 0:1], 0.0,
                                        ALU.add, ALU.max)
                qball[s] = qb

                nc.sync.dma_start(
                    contrib1[s][:].rearrange("t (c a w) -> (t c) a w", c=C, a=BR),
                    qb[:],
                )
                nc.gpsimd.collective_compute(
                    "AllGather", ALU.bypass, replica_groups=rg,
                    ins=[contrib1[s].opt()], outs=[gath1[s].opt()],
                )

            # ---------------- phase B ----------------
            ctxt = {}

            def phase_b_prep(s):
                ksb = kvp.tile([C, R, 3, PX], BF16, tag=f"ksb_{s}")
                nc.sync.dma_start(
                    ksb[:],
                    gath1[s][:].rearrange("g t (c px) -> c g t px", c=C),
                )
                vtsb = kvp.tile([128, 3, R, 4, 33], BF16, tag=f"vtsb_{s}")
                nc.vector.memset(vtsb[:, :, :, :, 32:33], 1.0)
                for t in (2, 0, 1):          # pass p consumes vt=(p+2)%3
                    for g in range(R):
                        trp = psum_m.tile([128, 4, C], BF16, tag="m", name="trp")
                        for ip in range(4):
                            nc.tensor.transpose(
                                trp[:, ip, :],
                                ksb[:, g, t, 128 * ip:128 * ip + 128],
                                ident[0:C, 0:C],
                            )
                        nc.vector.tensor_copy(vtsb[:, t, g, :, 0:32], trp[:])
                qrhs = kvp.tile([C, 3, PX], BF16, tag=f"qrhs_{s}")
                for t in range(3):
                    nc.vector.tensor_copy(
                        qrhs[:, t, :],
                        qball[s][32 * t:32 * t + 32, :].rearrange("c a w -> c (a w)"))
                return ksb, vtsb, qrhs

            def phase_b_unit(s, p, ksb, vtsb, qrhs_all):
                qt, kt, vt = p, (p + 1) % 3, (p + 2) % 3
                qrhs = qrhs_all[:, qt, :]

                ctxps = psum_m.tile([128, 4, 33], F32, tag="m", name="ctxps")
                # 4 interleaved accumulation streams share this bank: hardware
                # accumulates per-cell, but start=True would zero the whole
                # 2KB region — so pre-zero once and accumulate with start=False.
                nc.vector.memset(ctxps[:], 0.0)
                ngroups = (NCH + GS - 1) // GS

                def emit_s_group(g):
                    csz = min(GS, NCH - g * GS)
                    sps = psum_s.tile([128, GS, PX], F32, tag="s", name="sps")
                    for ci in range(csz):
                        i = g * GS + ci
                        rr, ip = divmod(i, 4)
                        nc.tensor.matmul(
                            sps[:, ci, :],
                            ksb[:, rr, kt, 128 * ip:128 * ip + 128],
                            qrhs,
                            start=True, stop=True,
                        )
                    return sps, csz

                sps, csz = emit_s_group(0)
                for g in range(ngroups):
                    es = ep.tile([128, GS, PX], BF16, tag="e")
                    nc.scalar.activation(
                        es[:].rearrange("p a w -> p (a w)")[:, 0:csz * PX],
                        sps[:].rearrange("p a w -> p (a w)")[:, 0:csz * PX],
                        AF.Exp, scale=SCALE,
                    )
                    cur_csz = csz
                    if g + 1 < ngroups:
                        sps, csz = emit_s_group(g + 1)
                    for ci in range(cur_csz):
                        i = g * GS + ci
                        rr, ip = divmod(i, 4)
                        for qb_ in range(4):
                            nc.tensor.matmul(
                                ctxps[:, qb_, :],
                                es[:, ci, 128 * qb_:128 * qb_ + 128],
                                vtsb[:, vt, rr, ip, :],
                                start=False, stop=(i == NCH - 1),
                                skip_group_check=True,
                            )

                # normalize per query partition, transpose back to ch-major
                rs = smallp.tile([128, 4], F32, tag="rs")
                nc.vector.tensor_copy(
                    rs[:], ctxps[:, :, 32:33].rearrange("p a o -> p (a o)"))
                recip = smallp.tile([128, 4], F32, tag="recip")
                nc.vector.reciprocal(recip[:], rs[:])
                ctxn = smallp.tile([128, 4, C], BF16, tag="ctxn")
                for qb_ in range(4):
                    nc.vector.tensor_scalar(
                        ctxn[:, qb_, :], ctxps[:, qb_, 0:32],
                        recip[:, qb_:qb_ + 1], None, ALU.mult,
                    )
                trc = psum_s.tile([C, 4, 128], BF16, tag="s", name="trc")
                for qb_ in range(4):
                    nc.tensor.transpose(
                        trc[:, qb_, :], ctxn[:, qb_, :], ident[:],
                    )
                nc.vector.tensor_copy(
                    ctxt[s][:, p, :, 1:65],
                    trc[:].rearrange("c a q -> c (a q)").rearrange(
                        "c (j w) -> c j w", w=W))

            def phase_b_sample(s, passes):
                for p in passes:
                    phase_b_unit(s, p, *_prep[s])

            _prep = {}
            for s in range(B):
                ctxt[s] = persistp.tile([C, 3, BR, 66], BF16, tag=f"ctxt_{s}",
                                        name=f"ctxt_{s}")
                nc.vector.memset(ctxt[s][:, :, :, 0:1], 0.0)
                nc.vector.memset(ctxt[s][:, :, :, 65:66], 0.0)

            _prep[0] = phase_b_prep(0)
            phase_b_sample(0, (0, 1, 2))
            nc.sync.dma_start(
                contrib2[0][:].rearrange("t (c jw) -> c t jw", c=C),
                ctxt[0][:].rearrange("c t j w -> c t (j w)"))
            nc.gpsimd.collective_compute(
                "AllGather", ALU.bypass, replica_groups=rg,
                ins=[contrib2[0].opt()], outs=[gath2[0].opt()],
            )
            _prep[1] = phase_b_prep(1)
            phase_b_sample(1, (0, 1, 2))
            nc.sync.dma_start(
                contrib2[1][:].rearrange("t (c jw) -> c t jw", c=C),
                ctxt[1][:].rearrange("c t j w -> c t (j w)"))
            nc.gpsimd.collective_compute(
                "AllGather", ALU.bypass, replica_groups=rg,
                ins=[contrib2[1].opt()], outs=[gath2[1].opt()],
            )

            # ---------------- phase C ----------------
            for s in range(B):
                # gathered ctx, pass-stacked on partitions, 3 guard rows each side
                cf = phcp.tile([96, 70, 66], BF16, tag=f"cf_{s}")
                nc.vector.memset(cf[:, 0:3, :], 0.0)
                nc.vector.memset(cf[:, 67:70, :], 0.0)
                nc.sync.dma_start(
                    cf[:, 3:67, :].rearrange("pc (g j) w -> pc g (j w)", g=R),
                    gath2[s][:].rearrange("g t (c jw) -> (t c) g jw", c=C),
                )

                xctx = phcp.tile([96, 12, 66], BF16, tag="xctx")
                nc.vector.memset(xctx[:, :, 0:1], 0.0)
                nc.vector.memset(xctx[:, :, 65:66], 0.0)
                nc.vector.tensor_copy(xctx[0:CIN, :, :], xband[:, s, :, :])  # 66-wide incl pads

                # wr|wg|wb as one block-diagonal 96-wide conv, two row halves
                tmp = phcp.tile([96, 12, W], BF16, tag="tmpC")
                for j0 in (0, 6):
                    ps = conv_psum([96, 6, W])
                    for ti, (dy, dx) in enumerate(TAPS):
                        nc.tensor.matmul(
                            ps[:],
                            wC[:, 3 * dy + dx, :],
                            cf[:, bass.ds(pid * BR + j0 + dy, 6), dx:dx + W],
                            start=(ti == 0), stop=(ti == 8),
                        )
                    relu_img(tmp[:, j0:j0 + 6, :], ps[:],
                             biasC[:, j0:j0 + 6, :], [96, 6, W], "tC")

                # sum the three 32-partition groups of tmp via a selection matmul
                for j0 in (0, 6):
                    aps = conv_psum([C, 6, W])
                    nc.tensor.matmul(aps[:], sel3[:], tmp[:, j0:j0 + 6, :],
                                     start=True, stop=True)
                    nc.vector.tensor_copy(xctx[64:96, j0:j0 + 6, 1:65], aps[:])

                w2buf = phcp.tile([C, 10, 66], BF16, tag="w2buf")
                nc.vector.memset(w2buf[:, :, 0:1], 0.0)
                nc.vector.memset(w2buf[:, :, 65:66], 0.0)
                for j0 in (0, 5):
                    ps = conv_psum([C, 5, W])
                    for ti, (dy, dx) in enumerate(TAPS):
                        nc.tensor.matmul(
                            ps[:],
                            w2[:, 3 * dy + dx, :],
                            xctx[:, j0 + dy:j0 + dy + 5, dx:dx + W],
                            start=(ti == 0), stop=(ti == 8),
                        )
                    relu_img(w2buf[:, j0:j0 + 5, 1:65], ps[:],
                             biasD[:, j0:j0 + 5, :], [C, 5, W], "tD")

                ps = conv_psum([C, BR, W])
                for ti, (dy, dx) in enumerate(TAPS):
                    nc.tensor.matmul(
                        ps[:],
                        w3[:, 3 * dy + dx, :],
                        w2buf[:, dy:dy + BR, dx:dx + W],
                        start=(ti == 0), stop=(ti == 8),
                    )
                outsb = smallp.tile([C, BR, W], F32, tag="outsb")
                nc.vector.tensor_scalar(outsb[:], ps[:], bias3[:, 0:1], 0.0,
                                        ALU.add, ALU.max)
                nc.sync.dma_start(out_d[s], outsb[:])

    nc.compile()
    return nc


def _pack_w(w):
    # [Cout, Cin, 3, 3] -> lhsT pack [Cin, 9, Cout]
    w = np.asarray(w, np.float32)
    return np.ascontiguousarray(w.transpose(1, 2, 3, 0).reshape(w.shape[1], 9, w.shape[0]))


NEG = np.float32(-1e30)


def _bf16(a):
    import ml_dtypes
    return np.asarray(a, np.float32).astype(ml_dtypes.bfloat16)


def prep_in_maps(inputs):
    x = np.asarray(inputs["x"], np.float32)
    xp = np.zeros((B, CIN, H + 4, W + 2), np.float32)
    xp[:, :, 2:2 + H, 1:1 + W] = x

    shared = {}
    # wA: conv1 stacked [64, 9, 96] (q|k|v)
    shared["wA"] = _bf16(np.concatenate(
        [_pack_w(inputs[n]) for n in ("wq1", "wk1", "wv1")], axis=2))
    # wB: conv2 block-diagonal [96, 9, 96]
    wBb = np.zeros((96, 9, 96), np.float32)
    for t, n in enumerate(("wq2", "wk2", "wv2")):
        wBb[32 * t:32 * t + 32, :, 32 * t:32 * t + 32] = _pack_w(inputs[n])
    shared["wB"] = _bf16(wBb)
    # wC: wr|wg|wb block-diagonal [96, 9, 96] bf16
    wCb = np.zeros((96, 9, 96), np.float32)
    for t, n in enumerate(("wr", "wg", "wb")):
        wCb[32 * t:32 * t + 32, :, 32 * t:32 * t + 32] = _pack_w(inputs[n])
    shared["wC"] = _bf16(wCb)
    w2v = np.asarray(inputs["w2"], np.float32).copy()
    w2v[:, CIN:, :, :] /= 3.0   # fold the ctx 3-way average into w2
    shared["w2"] = _bf16(_pack_w(w2v))
    shared["w3"] = _bf16(_pack_w(inputs["w3"]))
    shared["ident"] = _bf16(np.eye(128, dtype=np.float32))
    shared["sel3"] = _bf16(np.tile(np.eye(C, dtype=np.float32), (3, 1)))
    shared["biasQ"] = np.concatenate(
        [np.asarray(inputs[n], np.float32) for n in ("bq2", "bk2", "bv2")]
    ).reshape(96, 1)
    shared["bias3"] = np.asarray(inputs["b3"], np.float32).reshape(C, 1)

    bvals = {n: np.asarray(inputs[n], np.float32)
             for n in ("bq1", "bk1", "bv1", "br", "bg", "bb", "b2")}

    in_maps = []
    for r in range(R):
        r0 = BR * r
        xbandv = np.ascontiguousarray(
            xp[:, :, r0:r0 + 12, :].transpose(1, 0, 2, 3)
        )  # [CIN, B, 12, 66]

        # bias images; -1e30 rows get relu'd to the zero SAME padding expects
        biasA = np.concatenate(
            [np.broadcast_to(bvals[n][:, None, None], (C, 10, W)).copy()
             for n in ("bq1", "bk1", "bv1")], axis=0)   # [96, 10, W]
        biasCv = np.concatenate(
            [np.broadcast_to(bvals[n][:, None, None], (C, 12, W)).copy()
             for n in ("br", "bg", "bb")], axis=0)      # [96, 12, W]
        biasD = np.broadcast_to(bvals["b2"][:, None, None], (C, 10, W)).copy()
        if r == 0:
            biasA[:, 0, :] = NEG
            biasCv[:, 0:2, :] = NEG
            biasD[:, 0, :] = NEG
        if r == R - 1:
            biasA[:, 9, :] = NEG
            biasCv[:, 10:12, :] = NEG
            biasD[:, 9, :] = NEG

        in_maps.append(dict(
            shared, xband=_bf16(xbandv),
            biasA=np.ascontiguousarray(biasA),
            biasC=np.ascontiguousarray(biasCv),
            biasD=np.ascontiguousarray(biasD),
        ))
    return in_maps


_CACHE = {}


def get_program():
    if "nc" not in _CACHE:
        _CACHE["nc"] = build_program()
    return _CACHE["nc"]


def kernel(**inputs):
    nc = get_program()
    in_maps = prep_in_maps(inputs)
    res = run_bass_kernel_spmd(nc, in_maps, list(range(R)))
    out = np.zeros((B, C, H, W), np.float32)
    for r in range(R):
        out[:, :, BR * r:BR * (r + 1), :] = res.results[r]["out"]
    return out
